# revision 10
# baseline (speedup 1.0000x reference)
"""Soft-MoE forward on 8 TRN2 NeuronCores — v2 (SBUF-resident restructure).

Data-parallel over batch (B=16 -> 2 per core). Matmuls in bf16 except the
combine, which runs fp8e4 DoubleRow (half the instructions at 2x rate).
Full fp8 breaks the 2e-2 accuracy gate (quantization noise ~3.6%/tensor
does not average through random-sign contractions; measured 6.8e-2
all-fp8, 3.5e-2 for router/dispatch-fp8) but combine-only fp8 lands at
1.36e-2: PT is cast bf16->fp8 for free in the gpsimd store DMA, and eo is
scaled by 2^13 into fp8 range (descale folded into the rzc factor).

Structural changes vs v1:
- P = exp(router logits) stays resident in SBUF per batch (64KB/part);
  dispatch reads it directly (v1 re-read P twice from DRAM).
- P^T produced by SBUF->SBUF DMA transpose chunks, streamed to DRAM in a
  chunk-contiguous layout, prefetched during combine.
- eo (expert outputs) stay resident in SBUF, reusing the dead buffers of
  wrt (after router) and P (after dispatch). No eo DRAM round trip.
- eit (dispatch output) round-trips DRAM once (SBUF budget), 1KB runs.
- y written directly from PSUM (in-place rzc scale on DVE, then DMA).
- Engine split: loads on SP(sync) queue, transposes on ACT queue,
  stores on Pool(gpsimd) queue; PSUM->SBUF copies spread over
  ACT/DVE/Pool.
"""

import numpy as np
import ml_dtypes

import concourse.bass as bass
import concourse.tile as tile
from concourse import mybir

B, T, C, E, H = 16, 2048, 1024, 16, 2048
CAP = T // E  # 128
S = E * CAP  # 2048 slots
P = 128
NCORES = 8
BL = B // NCORES  # 2

Tt = T // P   # 16
Ct = C // P   # 8
St = S // P   # 16
Ht = H // P   # 16

FP32 = mybir.dt.float32
BF16 = mybir.dt.bfloat16
F8E4 = mybir.dt.float8e4
DR = mybir.MatmulPerfMode.DoubleRow
EOSC = 2.0 ** 13  # eo fp8 storage scale; folded out via rzc
AX = mybir.AluOpType
AF = mybir.ActivationFunctionType


def _split_multi_waits(nc):
    """This walrus build accepts only ONE sync wait per instruction; Tile's
    wait-assignment can emit several. Move extra waits onto single-wait nops
    inserted just before the instruction on the same engine."""
    import bass_rust

    nid = 0
    for f in nc.m.functions:
        for bb in f.blocks:
            out = []
            changed = False
            for inst in bb.instructions:
                si = inst.sync_info
                waits = list(si.on_wait) if si and si.on_wait else []
                if len(waits) > 1:
                    changed = True
                    for w in waits[:-1]:
                        nop = mybir.InstNoOp(name=f"TW-{nid}", ins=[], outs=[])
                        nid += 1
                        nop.engine = inst.engine
                        nop.sync_info = bass_rust.SyncInfo(on_wait=[w], on_update=[])
                        out.append(nop)
                    si.on_wait = waits[-1:]
                out.append(inst)
            if changed:
                bb.instructions = out


def build_nc(loops=BL, split_waits=True):
    assert loops % BL == 0
    nc = bass.Bass(trn_type="TRN2")

    xb = nc.dram_tensor("xb", [BL, T, C], BF16, kind="ExternalInput")
    xbt = nc.dram_tensor("xbt", [BL, C, T], BF16, kind="ExternalInput")
    wrt = nc.dram_tensor("wrt", [C, S], BF16, kind="ExternalInput")
    wg = nc.dram_tensor("wg", [E, C, H], BF16, kind="ExternalInput")
    wf = nc.dram_tensor("wf", [E, C, H], BF16, kind="ExternalInput")
    wp = nc.dram_tensor("wp", [E, H, C], BF16, kind="ExternalInput")
    y = nc.dram_tensor("y", [BL, T, C], FP32, kind="ExternalOutput")

    with tile.TileContext(nc) as tc:
        with (
            tc.tile_pool(name="dram", bufs=2, space="DRAM") as dpool,
            tc.tile_pool(name="big", bufs=1) as cpool,
            tc.tile_pool(name="st3", bufs=3) as p3,
            tc.tile_pool(name="st2", bufs=2) as p2,
            tc.tile_pool(name="stat", bufs=2) as sp,
            tc.tile_pool(name="psum", bufs=4, space="PSUM") as pp,
        ):
            # round-robin PSUM->SBUF copy engine to avoid single-queue tails
            def _copy(i, out, in_):
                # gpsimd cannot read PSUM (walrus birverifier)
                if i % 2 == 0:
                    nc.scalar.copy(out, in_)
                else:
                    nc.vector.tensor_copy(out, in_)

            for rep in range(loops // BL):
                # wrt resident; tag shared with eo0 (wrt dead after R of b=1).
                # Chunked load so router tt=0 matmuls start after chunk c=0.
                wrt_sb = cpool.tile([P, Ct, S], BF16, tag="a32", bufs=1, name="wrt_sb")
                for c in range(Ct):
                    eng = nc.scalar if c % 2 == 0 else nc.gpsimd
                    eng.dma_start(
                        wrt_sb[:, c], wrt.rearrange("(c p) s -> p c s", p=P)[:, c]
                    )

                pt_drams, rzcs, rzds = {}, {}, {}
                rzc2s = {}
                eit_drams = {}
                p_sbs = {}
                pre_e0 = {}
                for b in range(BL):
                    # ---- R+T: router logits, exp, transpose chunks ----
                    p_sb = cpool.tile([P, Tt, S], BF16, tag="big64", bufs=1,
                                      name=f"p_sb{b}")
                    p_sbs[b] = p_sb
                    pt_dram = dpool.tile([Tt, P, S], F8E4, tag="pt_dram",
                                         name=f"ptd{b}")
                    eit_dram = dpool.tile([P, Ct, S], BF16, tag="eit_dram",
                                          name=f"eitd{b}")
                    pt_drams[b], eit_drams[b] = pt_dram, eit_dram
                    rzc = sp.tile([P, Tt], FP32, tag="rzc", name=f"rzc{b}")
                    rzd = sp.tile([P, St], FP32, tag="rzd", name=f"rzd{b}")
                    rzcs[b], rzds[b] = rzc, rzd
                    zdall = sp.tile([P, St, Tt], FP32, tag="zdall", bufs=1,
                                    name="zdall")

                    for t in range(Tt):
                        xbt_t = p3.tile([P, Ct, P], BF16, tag="xbt_t", bufs=2,
                                        name="xbt_t")
                        nc.sync.dma_start(
                            xbt_t[:],
                            xbt[b].rearrange("(c p) t -> p c t", p=P)[
                                :, :, t * P : (t + 1) * P
                            ],
                        )
                        gps = [pp.tile([P, 512], FP32, tag="pa", name=f"gps{n}")
                               for n in range(4)]
                        for c in range(Ct):
                            for n in range(4):
                                nc.tensor.matmul(
                                    gps[n][:],
                                    xbt_t[:, c, :],
                                    wrt_sb[:, c, n * 512 : (n + 1) * 512],
                                    start=(c == 0),
                                    stop=(c == Ct - 1),
                                )
                        zc4 = sp.tile([P, 4], FP32, tag="zc4", bufs=3, name="zc4")
                        for n in range(4):
                            nc.scalar.activation(
                                p_sb[:, t, n * 512 : (n + 1) * 512],
                                gps[n][:],
                                AF.Exp,
                                accum_out=zc4[:, n : n + 1],
                            )
                        zc1 = sp.tile([P, 1], FP32, tag="zc1", bufs=4, name="zc1")
                        nc.vector.tensor_reduce(zc1[:], zc4[:], mybir.AxisListType.X, AX.add)
                        nc.vector.reciprocal(rzc[:, t : t + 1], zc1[:])

                        # transpose chunk t in two half-slabs (smaller
                        # staging; DMA engines overlap PE on t+1)
                        for sh in range(2):
                            ptc = p3.tile([P, St // 2, P], BF16, tag="ptc",
                                          bufs=2, name="ptc")
                            nc.scalar.dma_start_transpose(
                                ptc[:], p_sb[:, t, sh * 1024 : (sh + 1) * 1024]
                            )
                            nc.vector.tensor_reduce(
                                zdall[:, sh * 8 : (sh + 1) * 8, t], ptc[:],
                                mybir.AxisListType.X, AX.add
                            )
                            nc.gpsimd.dma_start(
                                pt_dram[t, :, sh * 1024 : (sh + 1) * 1024],
                                ptc[:],
                            )

                    zd = sp.tile([P, St], FP32, tag="zd", bufs=2, name="zd")
                    nc.vector.tensor_reduce(zd[:], zdall[:], mybir.AxisListType.X, AX.add)
                    nc.vector.reciprocal(rzd[:], zd[:])
                    rzc2 = sp.tile([P, Tt], FP32, tag="rzc2", name=f"rzc2{b}")
                    nc.vector.tensor_scalar_mul(rzc2[:], rzc[:], 1.0 / EOSC)
                    rzc2s[b] = rzc2

                    # ---- D: dispatch eit = x^T @ P (unnormalized) ----
                    # nh outer: slot-halves complete in order, so expert 0's
                    # loads (prefetched below) start while nh=1 still runs.
                    for nh in range(2):
                        if b == 1 and nh == 1:
                            pre_e0["eite"] = {}
                            for bb in range(BL):
                                eit_e = p2.tile([P, Ct, P], BF16, tag="eit_e",
                                                bufs=3, name=f"eite{bb}")
                                nc.sync.dma_start(
                                    eit_e[:],
                                    eit_drams[bb][:, :, 0:P],
                                )
                                pre_e0["eite"][bb] = eit_e
                            wgc = p2.tile([P, Ct, 512], BF16, tag="wgc",
                                          bufs=3, name="wgc")
                            nc.sync.dma_start(
                                wgc[:],
                                wg[0].rearrange("(c p) h -> p c h", p=P)[:, :, 0:512],
                            )
                            wfcc = p2.tile([P, Ct, 512], BF16, tag="wfcc",
                                           bufs=2, name="wfcc")
                            nc.gpsimd.dma_start(
                                wfcc[:],
                                wf[0].rearrange("(c p) h -> p c h", p=P)[:, :, 0:512],
                            )
                            pre_e0["w0"] = (wgc, wfcc)
                        for mh in range(2):
                            dps = [pp.tile([P, 512], FP32,
                                           tag=("pa" if i < 4 else "pb"),
                                           name=f"dps{i}")
                                   for i in range(8)]
                            for k in range(Tt):
                                xk = p3.tile([P, 512], BF16, tag="xk", bufs=2,
                                             name="xk")
                                nc.sync.dma_start(
                                    xk[:],
                                    xb[b].rearrange("(k p) c -> p k c", p=P)[
                                        :, k, mh * 512 : (mh + 1) * 512
                                    ],
                                )
                                for m4 in range(4):
                                    for n2 in range(2):
                                        nc.tensor.matmul(
                                            dps[m4 * 2 + n2][:],
                                            xk[:, m4 * P : (m4 + 1) * P],
                                            p_sb[:, k,
                                                 nh * 1024 + n2 * 512
                                                 : nh * 1024 + (n2 + 1) * 512],
                                            start=(k == 0),
                                            stop=(k == Tt - 1),
                                        )
                            for i in range(8):
                                m = mh * 4 + i // 2
                                n = nh * 2 + i % 2
                                est = p3.tile([P, 512], BF16, tag="ysb", bufs=4,
                                              name="est")
                                _copy(i, est[:], dps[i][:])
                                nc.gpsimd.dma_start(
                                    eit_dram[:, m, n * 512 : (n + 1) * 512],
                                    est[:],
                                )

                # ---- M: per-expert GLU MLP, software-pipelined: expert e's
                # gg/hh runs on PE while expert e-1's h transposes on the DMA
                # engines; e-1's eo matmuls then fill what would be the
                # transpose stall. eo -> SBUF resident: eo0 reuses wrt's
                # arena slot, eo1 reuses P's (dead after D of b=1).
                eo0_t = cpool.tile([P, St, C], F8E4, tag="a32", bufs=1, name="eo0")
                eo1_t = cpool.tile([P, St, C], F8E4, tag="big64", bufs=1, name="eo1")
                eos = {0: eo0_t, 1: eo1_t}

                def emit_prefetch(e, into):
                    into["eite"] = {}
                    for bb in range(BL):
                        eit_e = p2.tile([P, Ct, P], BF16, tag="eit_e",
                                        bufs=3, name=f"eite{bb}")
                        nc.sync.dma_start(
                            eit_e[:],
                            eit_drams[bb][:, :, e * P : (e + 1) * P],
                        )
                        into["eite"][bb] = eit_e
                    wgc = p2.tile([P, Ct, 512], BF16, tag="wgc", bufs=3,
                                  name="wgc")
                    nc.sync.dma_start(
                        wgc[:],
                        wg[e].rearrange("(c p) h -> p c h", p=P)[:, :, 0:512],
                    )
                    wfcc = p2.tile([P, Ct, 512], BF16, tag="wfcc", bufs=2,
                                   name="wfcc")
                    nc.scalar.dma_start(
                        wfcc[:],
                        wf[e].rearrange("(c p) h -> p c h", p=P)[:, :, 0:512],
                    )
                    into["w0"] = (wgc, wfcc)

                def emit_gghh(e, pre):
                    eites = pre["eite"]
                    hs = {
                        b: p2.tile([P, H], BF16, tag=f"hs{b}", bufs=1, name=f"hsb{b}")
                        for b in range(BL)
                    }
                    for hc in range(4):
                        if hc == 0:
                            wgc, wfcc = pre["w0"]
                        else:
                            wgc = p2.tile([P, Ct, 512], BF16, tag="wgc",
                                          bufs=3, name="wgc")
                            nc.sync.dma_start(
                                wgc[:],
                                wg[e].rearrange("(c p) h -> p c h", p=P)[
                                    :, :, hc * 512 : (hc + 1) * 512
                                ],
                            )
                            wfcc = p2.tile([P, Ct, 512], BF16, tag="wfcc",
                                           bufs=2, name="wfcc")
                            nc.scalar.dma_start(
                                wfcc[:],
                                wf[e].rearrange("(c p) h -> p c h", p=P)[
                                    :, :, hc * 512 : (hc + 1) * 512
                                ],
                            )
                        for b in range(BL):
                            gg = pp.tile([P, 512], FP32, tag="pa", name=f"gg{b}")
                            hh = pp.tile([P, 512], FP32, tag="pa", name=f"hh{b}")
                            for c in range(Ct):
                                # same stationary operand back-to-back: lets
                                # codegen/hw skip the redundant LDWEIGHTS
                                nc.tensor.matmul(
                                    gg[:], eites[b][:, c, :], wgc[:, c, :],
                                    start=(c == 0), stop=(c == Ct - 1),
                                )
                                nc.tensor.matmul(
                                    hh[:], eites[b][:, c, :], wfcc[:, c, :],
                                    start=(c == 0), stop=(c == Ct - 1),
                                )
                            sg = p3.tile([P, 512], BF16, tag="sg", bufs=1,
                                         name="sg")
                            nc.scalar.activation(
                                sg[:], gg[:], AF.Silu,
                                scale=rzds[b][:, e : e + 1],
                            )
                            nc.vector.scalar_tensor_tensor(
                                hs[b][:, hc * 512 : (hc + 1) * 512],
                                hh[:], rzds[b][:, e : e + 1], sg[:],
                                AX.mult, AX.mult,
                            )
                    hts = {}
                    for b in range(BL):
                        ht = p2.tile([P, Ht, P], BF16, tag="ht", bufs=3, name=f"htb{b}")
                        nc.scalar.dma_start_transpose(ht[:], hs[b][:])
                        hts[b] = ht
                    return hts

                def emit_eo(e, hts):
                    # wpc on the Pool (SWDGE) queue: keeps it out of the
                    # in-order sync queue behind the next expert's loads.
                    for cc in range(2):
                        eops = {}
                        for kh in range(2):
                            wpc = p2.tile([P, Ht // 2, 512], BF16, tag="wpc",
                                          name="wpc")
                            nc.gpsimd.dma_start(
                                wpc[:],
                                wp[e].rearrange("(k p) c -> p k c", p=P)[
                                    :, kh * 8 : (kh + 1) * 8,
                                    cc * 512 : (cc + 1) * 512
                                ],
                            )
                            for b in range(BL):
                                if kh == 0:
                                    eops[b] = pp.tile([P, 512], FP32, tag="pb",
                                                      name=f"eop{b}_{cc}")
                                for k in range(8):
                                    nc.tensor.matmul(
                                        eops[b][:], hts[b][:, kh * 8 + k, :],
                                        wpc[:, k, :],
                                        start=(kh == 0 and k == 0),
                                        stop=(kh == 1 and k == 7),
                                    )
                        for b in range(BL):
                            nc.vector.tensor_scalar_mul(
                                eos[b][:, e, cc * 512 : (cc + 1) * 512],
                                eops[b][:], EOSC,
                            )

                prev = None
                pre = pre_e0
                for e in range(E):
                    hts, pre_next = emit_gghh(e, pre), {}
                    if e + 1 < E:
                        emit_prefetch(e + 1, pre_next)
                    if prev is not None:
                        emit_eo(prev[0], prev[1])
                    prev = (e, hts)
                    pre = pre_next
                emit_eo(prev[0], prev[1])

                # ---- C: combine y = (P^T^T @ eo) * rzc ----
                for b in range(BL):
                    for t in range(Tt):
                        ptr = p3.tile([P, St, P], F8E4, tag="ptr", bufs=3,
                                      name="ptr")
                        nc.sync.dma_start(ptr[:], pt_drams[b][t])
                        ypss = [pp.tile([P, 512], FP32, tag="pa",
                                        name=f"yps{cc}") for cc in range(2)]
                        for e in range(0, St, 2):
                            for cc in range(2):
                                nc.tensor.matmul(
                                    ypss[cc][:],
                                    ptr[:, e : e + 2, :],
                                    eos[b][:, e : e + 2,
                                           cc * 512 : (cc + 1) * 512],
                                    start=(e == 0),
                                    stop=(e == St - 2),
                                    perf_mode=DR,
                                )
                        for cc in range(2):
                            ysb = p3.tile([P, 512], FP32, tag="ysb", bufs=4,
                                          name="ysb")
                            nc.vector.tensor_scalar_mul(
                                ysb[:], ypss[cc][:], rzc2s[b][:, t : t + 1]
                            )
                            nc.gpsimd.dma_start(
                                y[b, t * P : (t + 1) * P,
                                  cc * 512 : (cc + 1) * 512],
                                ysb[:],
                            )
    if split_waits:
        _split_multi_waits(nc)
    return nc


def make_in_maps(x, w_router_gate, w_fc, w_gate, w_proj):
    bf16 = ml_dtypes.bfloat16
    wrt_np = np.ascontiguousarray(w_router_gate.reshape(S, C).T).astype(bf16)
    wg_np = w_gate.astype(bf16)
    wf_np = w_fc.astype(bf16)
    wp_np = w_proj.astype(bf16)

    in_maps = []
    for c in range(NCORES):
        xc = x[c * BL : (c + 1) * BL]
        xb_np = xc.astype(bf16)
        xbt_np = np.ascontiguousarray(xb_np.transpose(0, 2, 1))
        in_maps.append(
            {"xb": xb_np, "xbt": xbt_np, "wrt": wrt_np,
             "wg": wg_np, "wf": wf_np, "wp": wp_np}
        )
    return in_maps


def kernel(x, w_router_gate, w_fc, w_gate, w_proj):
    in_maps = make_in_maps(x, w_router_gate, w_fc, w_gate, w_proj)

    from concourse.bass_utils import run_bass_kernel_spmd

    nc = build_nc()
    res = None
    last_err = None
    for attempt in range(4):
        try:
            res = run_bass_kernel_spmd(nc, in_maps, core_ids=list(range(NCORES)))
            break
        except Exception as e:  # transient NRT_EXEC_UNIT_UNRECOVERABLE on first exec
            last_err = e
            import time as _time

            _time.sleep(5)
    if res is None:
        raise last_err
    y = np.concatenate(
        [res.results[c]["y"] for c in range(NCORES)], axis=0
    ).astype(np.float32)
    return y


if __name__ == "__main__":
    print("built", build_nc())
